# revision 36
# baseline (speedup 1.0000x reference)
# GNN mean-aggregation kernel for Trainium2 (8 NeuronCores, SPMD).
#
# Computes: out[i] = (1/deg_i) * sum_{(i,j) in E} (x[j] @ W + b)
# using the algebraic identity  out = inv_deg * (A @ x) @ W + b*mask,
# so the dense linear layer runs on the 100k aggregated rows instead of
# per-edge features.
#
# Sharding: destination nodes (and their incoming edge rows -- `row` is
# sorted) are split contiguously across 8 cores; x and W are replicated,
# so no collectives are needed.
#
# Per-core pipeline (all on-chip data bf16 except PSUM/inv_deg/output fp32):
#   1. dma_gather (GPSIMD SWDGE) fetches x[col] rows (256B bf16 each) from
#      HBM in 1024-index calls.  int16 gather indices only span 32k rows, so
#      x is addressed in 4 chunks of 25k rows and edges are host-binned by
#      (dest-tile, chunk), padded to a fixed per-bin column count so the
#      single SPMD instruction stream fits every core.
#   2. DVE builds one-hot segment matrices S^T[e,d] = (rel[e]==d) from
#      host-provided relative-dest values via tensor_tensor(is_equal).
#   3. PE accumulates AGG^T = sum_j M_j^T @ S^T_j in PSUM per 128-dest
#      tile, then OUT^T = W^T @ AGG^T + b (x) deg  (rank-1 bias matmul).
#   4. DVE scales by inv_deg along the dest axis; DMA writes OUT^T.
# Host post-processing transposes and concatenates the per-core outputs.
#
# bf16 matters: fp32 matmuls run at 4 cycles/row with fast-weight-load
# disabled; bf16 runs 1 cycle/row, dropping PE busy from ~1.02ms to ~0.45ms.
# The kernel is then bound by the SWDGE gather path (Q7 descriptor
# emission ~5us/call + random-HBM SDMA reads ~165GB/s effective).
#
# A v2 (_build2) that stages x chunks in SBUF and gathers SBUF->SBUF is
# kept below for reference: it is correct but 2.7x slower end-to-end,
# because transposed SBUF-source gathers corrupt each other across SWDGE
# queues (shared Q7 XBAR) and so must serialize on one queue.

import math

import ml_dtypes
import numpy as np

P = 128
F = 128
BF16 = ml_dtypes.bfloat16


class _Cfg:
    def __init__(self, n_nodes, n_cores, n_chunks, group_tiles=8):
        self.NN = n_nodes
        self.NCORES = n_cores
        self.NDEST = n_nodes // n_cores
        self.NT = math.ceil(self.NDEST / P)
        self.NCH = n_chunks
        self.CH = math.ceil(n_nodes / n_chunks)
        assert self.CH <= 32768
        self.G = group_tiles


CFG = _Cfg(100000, 8, 4)

_BUILD_CACHE = {}


def _host_prep(cfg, x, row, col, W, b):
    NN, NCORES, NDEST, NT, NCH, CH = (
        cfg.NN, cfg.NCORES, cfg.NDEST, cfg.NT, cfg.NCH, cfg.CH)
    NE = row.shape[0]
    row = np.asarray(row).astype(np.int64)
    col = np.asarray(col).astype(np.int64)
    x = np.ascontiguousarray(np.asarray(x, dtype=np.float32))
    W = np.ascontiguousarray(np.asarray(W, dtype=np.float32))
    b = np.asarray(b, dtype=np.float32)

    deg = np.bincount(row, minlength=NN).astype(np.float32)
    invdeg = np.where(deg > 0, 1.0 / np.maximum(deg, 1.0), 0.0).astype(np.float32)

    core = row // NDEST
    r_in_core = row % NDEST
    chunk = col // CH
    idx16 = (col % CH).astype(np.int16)

    # Natural (contiguous) dest->tile assignment unless some (tile, chunk)
    # bin would push C_sub above 9 columns; then greedily rebalance.
    nat_tile = r_in_core // P
    nat_key = (core * NT + nat_tile) * NCH + chunk
    nat_max = np.bincount(nat_key, minlength=NCORES * NT * NCH).max()
    if nat_max <= 9 * P:
        perm = np.tile(np.arange(NDEST, dtype=np.int64)[None, :], (NCORES, 1))
        tilei = nat_tile
        rel = (r_in_core % P).astype(np.float32)
        return _host_prep_finish(
            cfg, x, W, b, deg, invdeg, core, chunk, idx16, tilei, rel, perm)
    # perm[core, d_local] = permuted position (tile*128 + slot).
    perm = np.zeros((NCORES, NDEST), np.int64)
    for c in range(NCORES):
        cnt = np.zeros((NDEST, NCH), np.int32)
        np.add.at(cnt, (r_in_core[core == c], chunk[core == c]), 1)
        order_d = np.argsort(-cnt.max(axis=1), kind="stable")
        sums = np.zeros((NT, NCH), np.int32)
        counts = np.zeros(NT, np.int32)
        pos = np.empty(NDEST, np.int64)
        big = np.int32(1 << 30)
        for d in order_d:
            newmax = np.maximum(sums, cnt[d]).max(axis=1)
            t = int(np.argmin(np.where(counts < P, newmax, big)))
            pos[d] = t * P + counts[t]
            counts[t] += 1
            sums[t] += cnt[d]
        perm[c] = pos
    tilei = perm[core, r_in_core] // P
    rel = (perm[core, r_in_core] % P).astype(np.float32)
    return _host_prep_finish(
        cfg, x, W, b, deg, invdeg, core, chunk, idx16, tilei, rel, perm)


def _host_prep_finish(cfg, x, W, b, deg, invdeg, core, chunk, idx16,
                      tilei, rel, perm):
    NN, NCORES, NDEST, NT, NCH, CH = (
        cfg.NN, cfg.NCORES, cfg.NDEST, cfg.NT, cfg.NCH, cfg.CH)
    NE = core.shape[0]
    bin_key = (core * NT + tilei) * NCH + chunk
    nbins = NCORES * NT * NCH
    counts = np.bincount(bin_key, minlength=nbins)
    C_sub = max(1, int(math.ceil(counts.max() / P)))
    SLOT = C_sub * P

    # Within each bin, order edges by source index so the gather's HBM
    # addresses ascend monotonically per call (DRAM row-buffer locality).
    # Slot order inside a bin is free: the segment-sum is order-invariant
    # and rel follows the same permutation.
    order = np.lexsort((idx16, bin_key))
    sk = bin_key[order]
    starts = np.concatenate([[0], np.cumsum(counts)[:-1]])
    rank = np.arange(NE, dtype=np.int64) - starts[sk]
    pos = sk * SLOT + rank

    TOT = nbins * SLOT
    idx_pad = np.zeros(TOT, np.int16)
    rel_pad = np.full(TOT, -1.0, np.float32)
    idx_pad[pos] = idx16[order]
    rel_pad[pos] = rel[order]
    idx_pad = idx_pad.reshape(NCORES, NT, NCH, SLOT)
    rel_pad = rel_pad.reshape(NCORES, NT, NCH, C_sub, P)

    groups = [(t0, min(t0 + cfg.G, NT)) for t0 in range(0, NT, cfg.G)]

    iota2 = np.tile(np.arange(P, dtype=BF16)[None, :], (P, 1))
    brow = b[None, :].astype(BF16)
    x16 = x.astype(BF16)
    W16 = W.astype(BF16)

    in_maps = []
    for c in range(NCORES):
        # gather-call index stream: per (group, chunk), wrapped per <=1024-idx call
        wrapped_parts = []
        for (t0, t1) in groups:
            for ch in range(NCH):
                seq = idx_pad[c, t0:t1, ch].reshape(-1)
                for k0 in range(0, len(seq), 768):
                    seg = seq[k0:k0 + 768]
                    wrapped_parts.append(
                        np.tile(seg.reshape(-1, 16).T, (8, 1)))
        idx_t = np.concatenate(wrapped_parts, axis=1)

        rel_t = np.ascontiguousarray(
            rel_pad[c].transpose(3, 0, 1, 2).reshape(P, NT * NCH * C_sub)
        ).astype(BF16)

        dsl = slice(c * NDEST, (c + 1) * NDEST)
        ivc = np.zeros(NT * P, np.float32)
        ivc[perm[c]] = invdeg[dsl]
        dgc = np.zeros(NT * P, np.float32)
        dgc[perm[c]] = deg[dsl]

        in_maps.append({
            "x": x16,
            "idxs": np.ascontiguousarray(idx_t),
            "rel": rel_t,
            "invdeg": np.ascontiguousarray(np.tile(ivc[None, :], (P, 1))),
            "degr": dgc[None, :].astype(BF16),
            "w": W16,
            "brow": brow,
            "iota2": iota2,
        })
    return C_sub, in_maps, perm


def _build(cfg, C_sub, repeat, parts=("gather", "onehot", "mm")):
    import concourse.mybir as mybir
    import concourse.tile as tile
    from concourse import bacc

    f32 = mybir.dt.float32
    bf16 = mybir.dt.bfloat16
    i16 = mybir.dt.int16
    eq = mybir.AluOpType.is_equal
    mult = mybir.AluOpType.mult

    NT, NCH, CH, G = cfg.NT, cfg.NCH, cfg.CH, cfg.G
    C_tot = NCH * C_sub
    IDXW = NT * C_tot * P // 16

    nc = bacc.Bacc("TRN2", debug=False, num_swdge_queues=4)
    x_d = nc.dram_tensor("x", [cfg.NN, F], bf16, kind="ExternalInput")
    idx_d = nc.dram_tensor("idxs", [P, IDXW], i16, kind="ExternalInput")
    rel_d = nc.dram_tensor("rel", [P, NT * C_tot], bf16, kind="ExternalInput")
    invdeg_d = nc.dram_tensor("invdeg", [P, NT * P], f32, kind="ExternalInput")
    deg_d = nc.dram_tensor("degr", [1, NT * P], bf16, kind="ExternalInput")
    w_d = nc.dram_tensor("w", [F, F], bf16, kind="ExternalInput")
    b_d = nc.dram_tensor("brow", [1, F], bf16, kind="ExternalInput")
    iota_d = nc.dram_tensor("iota2", [P, P], bf16, kind="ExternalInput")
    out_d = nc.dram_tensor("outT", [P, NT * P], f32, kind="ExternalOutput")

    groups = [(t0, min(t0 + G, NT)) for t0 in range(0, NT, G)]
    x_ap = x_d.ap()

    with tile.TileContext(nc) as tc:
        with (
            tc.tile_pool(name="const", bufs=1) as constp,
            tc.tile_pool(name="reg", bufs=6) as regionp,
            tc.tile_pool(name="st", bufs=12) as stp,
            tc.tile_pool(name="idx", bufs=6) as idxp,
            tc.tile_pool(name="small", bufs=8) as smallp,
            tc.tile_pool(name="grp", bufs=3) as grpp,
            tc.tile_pool(name="acc", bufs=8, space="PSUM") as accp,
        ):
            w_sb = constp.tile([F, F], bf16)
            nc.sync.dma_start(w_sb[:], w_d.ap())
            b_sb = constp.tile([1, F], bf16)
            nc.sync.dma_start(b_sb[:], b_d.ap())
            iota_sb = constp.tile([P, P], bf16)
            nc.sync.dma_start(iota_sb[:], iota_d.ap())
            rel_sb = constp.tile([P, NT * C_tot], bf16)
            nc.sync.dma_start(rel_sb[:], rel_d.ap())

            def body(_iv=None):
                idx_off = 0
                qn = 0
                for (t0, t1) in groups:
                    gt = t1 - t0
                    invdeg_g = grpp.tile([P, gt * P], f32, tag="invdeg")
                    nc.sync.dma_start(
                        invdeg_g[:], invdeg_d.ap()[:, t0 * P:t1 * P])
                    deg_g = grpp.tile([1, gt * P], bf16, tag="deg")
                    nc.sync.dma_start(deg_g[:], deg_d.ap()[:, t0 * P:t1 * P])
                    accs = [
                        accp.tile([P, P], f32, tag="acc", name=f"acc{t0}_{k}")
                        for k in range(gt)
                    ]
                    for c in range(NCH):
                        ncols = gt * C_sub
                        reg = regionp.tile([P, ncols, P], bf16, tag="reg")
                        idxt = idxp.tile([P, ncols * 8], i16, tag="idx")
                        nc.sync.dma_start(
                            idxt[:], idx_d.ap()[:, idx_off:idx_off + ncols * 8])
                        idx_off += ncols * 8
                        for k0 in range(0, ncols, 6) if "gather" in parts else []:
                            kc = min(6, ncols - k0)
                            L = kc * P
                            nc.gpsimd.dma_gather(
                                out_ap=reg[:, k0:k0 + kc, :],
                                in_ap=x_ap[c * CH:min((c + 1) * CH, cfg.NN), :],
                                idxs_ap=idxt[:, k0 * 8:k0 * 8 + kc * 8],
                                num_idxs=L,
                                num_idxs_reg=L,
                                elem_size=F,
                                queue_num=qn % 4,
                            )
                            qn += 1
                        for ti in range(gt) if ("onehot" in parts or "mm" in parts) else []:
                            t = t0 + ti
                            st = stp.tile([P, C_sub, P], bf16, tag="st")
                            rel_sl = rel_sb[:, (t * NCH + c) * C_sub:
                                            (t * NCH + c + 1) * C_sub]
                            if "onehot" in parts:
                                nc.vector.tensor_tensor(
                                    out=st[:],
                                    in0=iota_sb[:].unsqueeze(1).to_broadcast(
                                        [P, C_sub, P]),
                                    in1=rel_sl.to_broadcast([P, C_sub, P]),
                                    op=eq,
                                )
                            accap = accs[ti][:]
                            for j in range(C_sub) if "mm" in parts else []:
                                nc.tensor.matmul(
                                    out=accap,
                                    lhsT=reg[:, ti * C_sub + j, :],
                                    rhs=st[:, j, :],
                                    start=(c == 0 and j == 0),
                                    stop=(c == NCH - 1 and j == C_sub - 1),
                                )
                    for ti in range(gt) if "mm" in parts else []:
                        t = t0 + ti
                        accap = accs[ti][:]
                        aggT = smallp.tile([P, P], bf16, tag="agg")
                        nc.scalar.copy(aggT[:], accap)
                        # reuse the same PSUM bank for the output matmul
                        nc.tensor.matmul(out=accap, lhsT=w_sb[:], rhs=aggT[:],
                                         start=True, stop=False)
                        nc.tensor.matmul(out=accap, lhsT=b_sb[:1, :],
                                         rhs=deg_g[:1, ti * P:(ti + 1) * P],
                                         start=False, stop=True)
                        osb = smallp.tile([P, P], f32, tag="osb")
                        nc.vector.tensor_tensor(
                            out=osb[:], in0=accap,
                            in1=invdeg_g[:, ti * P:(ti + 1) * P], op=mult)
                        nc.sync.dma_start(
                            out_d.ap()[:, t * P:(t + 1) * P], osb[:])

            if repeat == 1:
                body()
            else:
                with tc.For_i(0, repeat, 1) as iv:
                    body(iv)

    nc.compile()
    return nc


def _run(cfg, x, row, col, W, b, repeat=1, core_ids=None):
    from concourse import bass_utils

    C_sub, in_maps, perm = _host_prep(cfg, x, row, col, W, b)
    key = (cfg.NN, cfg.NCORES, C_sub, repeat)
    if key not in _BUILD_CACHE:
        _BUILD_CACHE[key] = _build(cfg, C_sub, repeat)
    nc = _BUILD_CACHE[key]
    if core_ids is None:
        core_ids = list(range(cfg.NCORES))
    res = bass_utils.run_bass_kernel_spmd(nc, in_maps, core_ids=core_ids)
    outs = []
    for c in range(len(core_ids)):
        outT = res.results[c]["outT"]
        outs.append(outT.T[perm[c]])
    return np.concatenate(outs, axis=0)


# ---------------------------------------------------------------------------
# v2: SBUF-staged gather.
#
# The v1 HBM dma_gather is request-limited (~43ns/descriptor/engine: random
# 256-512B HBM reads), capping the kernel at ~1.3ms.  v2 instead stages x
# chunks into SBUF with large sequential DMAs (25.7MB/core total) and runs
# the per-edge gather SBUF->SBUF, where small descriptors keep near-fabric
# bandwidth.  SBUF-source gathers write TRANSPOSED tiles (features on
# partitions), so a first PE pass multiplies each gathered 128-slot column by
# W (lhsT=gathered column, rhs=W), which simultaneously projects the messages
# and restores slot-major orientation; a second PE pass does the one-hot
# segment-sum.  Chunk loop is outermost; per-tile partial aggregates
# accumulate in a 6.4MB SBUF tensor across the 4 chunks.
#
#   x_staged[c][p][r] = x_bf16[c*25088 + r*128 + p]   (gather token layout)
#   pass1: Hp[slots,fout] = regT[:,col]^T @ W      (PSUM, copy->SBUF bf16)
#   pass2: acc[dest,fout] += st[:,col]^T @ Hp      (+ b*deg bias on chunk 0)
#   out[dest] = invdeg[dest] * acc


class _Cfg2:
    def __init__(self, n_nodes, n_cores, n_chunks):
        self.NN = n_nodes
        self.NCORES = n_cores
        self.NDEST = n_nodes // n_cores
        self.NT = math.ceil(self.NDEST / P)
        self.NCH = n_chunks
        self.CH = math.ceil(n_nodes / n_chunks / P) * P  # rank-aligned
        assert self.CH <= 32768
        self.RANKS = self.CH // P


CFG2 = _Cfg2(100000, 8, 4)


def _host_prep2(cfg, x, row, col, W, b, min_csub=1):
    NN, NCORES, NDEST, NT, NCH, CH, RANKS = (
        cfg.NN, cfg.NCORES, cfg.NDEST, cfg.NT, cfg.NCH, cfg.CH, cfg.RANKS)
    NE = row.shape[0]
    row = np.asarray(row).astype(np.int64)
    col = np.asarray(col).astype(np.int64)
    x = np.ascontiguousarray(np.asarray(x, dtype=np.float32))
    W = np.ascontiguousarray(np.asarray(W, dtype=np.float32))
    b = np.asarray(b, dtype=np.float32)

    deg = np.bincount(row, minlength=NN).astype(np.float32)
    invdeg = np.where(deg > 0, 1.0 / np.maximum(deg, 1.0), 0.0).astype(np.float32)

    core = row // NDEST
    r_in_core = row % NDEST
    chunk = col // CH
    idx16 = (col % CH).astype(np.int16)

    nat_tile = r_in_core // P
    nat_key = (core * NT + nat_tile) * NCH + chunk
    nat_max = np.bincount(nat_key, minlength=NCORES * NT * NCH).max()
    if nat_max <= 9 * P:
        perm = np.tile(np.arange(NDEST, dtype=np.int64)[None, :], (NCORES, 1))
        tilei = nat_tile
        rel = (r_in_core % P).astype(np.float32)
    else:
        perm = np.zeros((NCORES, NDEST), np.int64)
        for c in range(NCORES):
            cnt = np.zeros((NDEST, NCH), np.int32)
            np.add.at(cnt, (r_in_core[core == c], chunk[core == c]), 1)
            order_d = np.argsort(-cnt.max(axis=1), kind="stable")
            sums = np.zeros((NT, NCH), np.int32)
            counts = np.zeros(NT, np.int32)
            pos = np.empty(NDEST, np.int64)
            big = np.int32(1 << 30)
            for d in order_d:
                newmax = np.maximum(sums, cnt[d]).max(axis=1)
                t = int(np.argmin(np.where(counts < P, newmax, big)))
                pos[d] = t * P + counts[t]
                counts[t] += 1
                sums[t] += cnt[d]
            perm[c] = pos
        tilei = perm[core, r_in_core] // P
        rel = (perm[core, r_in_core] % P).astype(np.float32)

    bin_key = (core * NT + tilei) * NCH + chunk
    nbins = NCORES * NT * NCH
    counts = np.bincount(bin_key, minlength=nbins)
    C_sub = max(1, int(math.ceil(counts.max() / P)))
    SLOT = C_sub * P

    order = np.argsort(bin_key, kind="stable")
    sk = bin_key[order]
    starts = np.concatenate([[0], np.cumsum(counts)[:-1]])
    rank = np.arange(NE, dtype=np.int64) - starts[sk]
    pos = sk * SLOT + rank

    TOT = nbins * SLOT
    idx_pad = np.zeros(TOT, np.int16)
    rel_pad = np.full(TOT, -1.0, np.float32)
    idx_pad[pos] = idx16[order]
    rel_pad[pos] = rel[order]
    idx_pad = idx_pad.reshape(NCORES, NT, NCH, SLOT)
    rel_pad = rel_pad.reshape(NCORES, NT, NCH, C_sub, P)

    iota2 = np.tile(np.arange(P, dtype=BF16)[None, :], (P, 1))
    brow = b[None, :].astype(BF16)
    W16 = W.astype(BF16)

    # staged x: token i of chunk c at partition i%128, rank i//128
    xpad = np.zeros((NCH * CH, F), np.float32)
    xpad[:NN] = x
    x_staged = np.ascontiguousarray(
        xpad.reshape(NCH, RANKS, P, F).transpose(0, 2, 1, 3)
        .reshape(NCH * P, RANKS * F)).astype(BF16)

    in_maps = []
    for c in range(NCORES):
        # gather index stream: chunk-outer, tile-inner, <=896-idx calls
        # (transpose-RX ucode: num_idxs/16+2 descriptors must stay <= 64)
        wrapped_parts = []
        for ch in range(NCH):
            for t in range(NT):
                seq = idx_pad[c, t, ch]
                for k0 in range(0, len(seq), 896):
                    seg = seq[k0:k0 + 896]
                    wrapped_parts.append(
                        np.tile(seg.reshape(-1, 16).T, (8, 1)))
        idx_t = np.concatenate(wrapped_parts, axis=1)

        rel_t = np.ascontiguousarray(
            rel_pad[c].transpose(3, 0, 1, 2).reshape(P, NT * NCH * C_sub)
        ).astype(BF16)

        dsl = slice(c * NDEST, (c + 1) * NDEST)
        ivc = np.zeros(NT * P, np.float32)
        ivc[perm[c]] = invdeg[dsl]
        dgc = np.zeros(NT * P, np.float32)
        dgc[perm[c]] = deg[dsl]

        in_maps.append({
            "xs": x_staged,
            "idxs": np.ascontiguousarray(idx_t),
            "rel": rel_t,
            "ivd2": np.ascontiguousarray(ivc.reshape(NT, P).T),
            "degr": dgc[None, :].astype(BF16),
            "w": W16,
            "brow": brow,
            "iota2": iota2,
        })
    return C_sub, in_maps, perm


def _build2(cfg, C_sub, repeat, parts=("gather", "onehot", "proj", "mm")):
    import concourse.mybir as mybir
    import concourse.tile as tile
    from concourse import bacc

    f32 = mybir.dt.float32
    bf16 = mybir.dt.bfloat16
    i16 = mybir.dt.int16
    eq = mybir.AluOpType.is_equal
    mult = mybir.AluOpType.mult
    add = mybir.AluOpType.add

    # Transposed SBUF-source gathers share the Q7 XBAR: concurrent calls on
    # different SWDGE queues corrupt each other, so keep them on one queue.
    import os
    NQ = int(os.environ.get("V2_NQ", "1"))
    NT, NCH, CH, RANKS = cfg.NT, cfg.NCH, cfg.CH, cfg.RANKS
    SLOT = C_sub * P
    C_tot = NCH * C_sub

    nc = bacc.Bacc("TRN2", debug=False, num_swdge_queues=4)
    xs_d = nc.dram_tensor("xs", [NCH * P, RANKS * F], bf16, kind="ExternalInput")
    idx_d = nc.dram_tensor("idxs", [P, NCH * NT * SLOT // 16], i16,
                           kind="ExternalInput")
    rel_d = nc.dram_tensor("rel", [P, NT * C_tot], bf16, kind="ExternalInput")
    ivd_d = nc.dram_tensor("ivd2", [P, NT], f32, kind="ExternalInput")
    deg_d = nc.dram_tensor("degr", [1, NT * P], bf16, kind="ExternalInput")
    w_d = nc.dram_tensor("w", [F, F], bf16, kind="ExternalInput")
    b_d = nc.dram_tensor("brow", [1, F], bf16, kind="ExternalInput")
    iota_d = nc.dram_tensor("iota2", [P, P], bf16, kind="ExternalInput")
    out_d = nc.dram_tensor("out", [NT * P, F], f32, kind="ExternalOutput")

    with tile.TileContext(nc) as tc:
        with (
            tc.tile_pool(name="const", bufs=1) as constp,
            tc.tile_pool(name="xs", bufs=1) as xsp,
            tc.tile_pool(name="regT", bufs=3) as regp,
            tc.tile_pool(name="st", bufs=3) as stp,
            tc.tile_pool(name="hs", bufs=3) as hsp,
            tc.tile_pool(name="idx", bufs=3) as idxp,
            tc.tile_pool(name="osb", bufs=4) as osbp,
            tc.tile_pool(name="hp", bufs=4, space="PSUM") as hpp,
            tc.tile_pool(name="acc2", bufs=4, space="PSUM") as acc2p,
        ):
            w_sb = constp.tile([F, F], bf16)
            nc.sync.dma_start(w_sb[:], w_d.ap())
            b_sb = constp.tile([1, F], bf16)
            nc.sync.dma_start(b_sb[:], b_d.ap())
            iota_sb = constp.tile([P, P], bf16)
            nc.sync.dma_start(iota_sb[:], iota_d.ap())
            rel_sb = constp.tile([P, NT * C_tot], bf16)
            nc.sync.dma_start(rel_sb[:], rel_d.ap())
            ivd_sb = constp.tile([P, NT], f32)
            nc.sync.dma_start(ivd_sb[:], ivd_d.ap())
            deg_sb = constp.tile([1, NT * P], bf16)
            nc.sync.dma_start(deg_sb[:], deg_d.ap())
            acc_sb = constp.tile([P, NT * F], f32)

            def body(_iv=None):
                idx_off = 0
                qn = 0
                cp = 0
                for c in range(NCH):
                    xs = xsp.tile([P, RANKS * F], bf16, tag="xs")
                    nc.sync.dma_start(xs[:], xs_d.ap()[c * P:(c + 1) * P, :])
                    for t in range(NT):
                        idxt = idxp.tile([P, SLOT // 16], i16, tag="idx")
                        nc.sync.dma_start(
                            idxt[:], idx_d.ap()[:, idx_off:idx_off + SLOT // 16])
                        idx_off += SLOT // 16
                        regT = regp.tile([P, 1, SLOT], bf16, tag="regT")
                        for k0 in [] if "gather" not in parts else range(0, SLOT, 896):
                            kc = min(896, SLOT - k0)
                            nc.gpsimd.dma_gather(
                                out_ap=regT[:, :, k0:k0 + kc],
                                in_ap=xs[:],
                                idxs_ap=idxt[:, k0 // 16:(k0 + kc) // 16],
                                num_idxs=kc,
                                num_idxs_reg=kc,
                                elem_size=F,
                                transpose=True,
                                sbuf_tokens_per_rank=P,
                                sbuf_free_dim_per_rank=F * 2,
                                queue_num=qn % NQ,
                            )
                            qn += 1
                        st = stp.tile([P, C_sub, P], bf16, tag="st")
                        rel_sl = rel_sb[:, (t * NCH + c) * C_sub:
                                        (t * NCH + c + 1) * C_sub]
                        if "onehot" in parts:
                            nc.vector.tensor_tensor(
                                out=st[:],
                                in0=iota_sb[:].unsqueeze(1).to_broadcast(
                                    [P, C_sub, P]),
                                in1=rel_sl.to_broadcast([P, C_sub, P]),
                                op=eq,
                            )
                        # pass 1: project gathered columns through W
                        hs = hsp.tile([P, C_sub, P], bf16, tag="hs")
                        for j0 in [] if "proj" not in parts else range(0, C_sub, 4):
                            jc = min(4, C_sub - j0)
                            hp = hpp.tile([P, 4, P], f32, tag="hp")
                            for j in range(j0, j0 + jc):
                                nc.tensor.matmul(
                                    out=hp[:, j - j0, :],
                                    lhsT=regT[:, 0, j * P:(j + 1) * P],
                                    rhs=w_sb[:],
                                    start=(j == j0), stop=(j == j0 + jc - 1),
                                    skip_group_check=True,
                                )
                            if cp % 2 == 0:
                                nc.scalar.copy(hs[:, j0:j0 + jc, :],
                                               hp[:, :jc, :])
                            else:
                                nc.vector.tensor_copy(hs[:, j0:j0 + jc, :],
                                                      hp[:, :jc, :])
                            cp += 1
                        if "mm" not in parts:
                            continue
                        # pass 2: one-hot segment sum (+ bias on chunk 0)
                        acc2 = acc2p.tile([P, P], f32, tag="acc2")
                        for j in range(C_sub):
                            nc.tensor.matmul(
                                out=acc2[:],
                                lhsT=st[:, j, :],
                                rhs=hs[:, j, :],
                                start=(j == 0),
                                stop=(j == C_sub - 1 and c != 0),
                            )
                        if c == 0:
                            nc.tensor.matmul(
                                out=acc2[:],
                                lhsT=deg_sb[:1, t * P:(t + 1) * P],
                                rhs=b_sb[:1, :],
                                start=False, stop=True,
                            )
                        asl = acc_sb[:, t * F:(t + 1) * F]
                        if c == 0:
                            nc.vector.tensor_copy(asl, acc2[:])
                        else:
                            nc.vector.tensor_tensor(
                                out=asl, in0=acc2[:], in1=asl, op=add)
                        if c == NCH - 1:
                            osb = osbp.tile([P, 1, P], f32, tag="osb")
                            nc.vector.tensor_tensor(
                                out=osb[:],
                                in0=asl.unsqueeze(1),
                                in1=ivd_sb[:, t:t + 1].to_broadcast([P, 1, P]),
                                op=mult,
                            )
                            nc.sync.dma_start(
                                out_d.ap()[t * P:(t + 1) * P, :],
                                osb[:, 0, :])

            if repeat == 1:
                body()
            else:
                with tc.For_i(0, repeat, 1) as iv:
                    body(iv)

    nc.compile()
    return nc


def _run2(cfg, x, row, col, W, b, repeat=1, core_ids=None):
    from concourse import bass_utils

    C_sub, in_maps, perm = _host_prep2(cfg, x, row, col, W, b)
    key = ("v2", cfg.NN, cfg.NCORES, C_sub, repeat)
    if key not in _BUILD_CACHE:
        _BUILD_CACHE[key] = _build2(cfg, C_sub, repeat)
    nc = _BUILD_CACHE[key]
    if core_ids is None:
        core_ids = list(range(cfg.NCORES))
    res = bass_utils.run_bass_kernel_spmd(nc, in_maps, core_ids=core_ids)
    outs = []
    for c in range(len(core_ids)):
        out_arr = res.results[c]["out"]
        outs.append(out_arr[perm[c]])
    return np.concatenate(outs, axis=0)


def kernel(x, row, col, W, b):
    return _run2(CFG2, x, row, col, W, b, repeat=1)

